# revision 58
# baseline (speedup 1.0000x reference)
"""Trainium2 Bass kernel for nn_BinaryPathEncoder — v2.

Math: 6+5+5 bit-chunk table decomposition.  maps[u] = A(u&63) @ B @ C where
A is the low-6-bit chunk (transposed table rt), B bits 6-10, C bits 11-15
(shared table rn).  Per position: mm1 = (A@B)^T (deduped over distinct
(A,B) keys, arbitrary group sizes), mm2 = out = (A@B) @ C.

v2 over baseline:
 - mm1 dedup with arbitrary group sizes (G ~= 163/core vs 190)
 - rt table per-core dense slots: only ~10 of 64 entries built per core
   (positions are sorted by (A,B) key so each core sees a narrow A band)
 - PE warmup matmuls during input-DMA head (HAM warm before expm)
 - expm scaling-squaring s=2 (one fewer squaring round)
 - transposes wait on DMA sem directly (not the DVE ident chain)
 - output copies 4-positions wide on ACT (amortize fixed overhead),
   output DMA per quad; redundant semaphore waits elided
"""

import contextlib
import numpy as np

DIM = 256
NCORES = 8
P = 128

ENT = 512          # f16 elems per table entry per partition (2*DIM)
NAT_E = 63         # rn: codes 1..63 (5-bit variable chunk products)
NAT_STRIDE = NAT_E * ENT

NSTAGE = 4         # mm1 psum banks (psA)
NSX = 16           # stag slots
LAG = 4            # min groups of mm1 lead before a group's mm2s start
DEADLINE = 16      # group h's mm2s must be emitted before mm1(h+DEADLINE)
EXPM_S = 2
EXPM_N = 4
WARMUP_MM = 20

_NC_CACHE = {}
LAST_RESULTS = None


class _WaitTracker:
    """Elide semaphore waits whose threshold is already guaranteed."""

    def __init__(self):
        self.guar = {}

    def wait(self, eng, sem, thr):
        if thr <= 0:
            return
        if self.guar.get(id(sem), -1) >= thr:
            return
        eng.wait_ge(sem, thr)
        self.guar[id(sem)] = thr


def _build_nc(npos, cores, G, RT, debug=False):
    """cores: per-core dict with keys:
         rt_par: [RT] parent rn code for each dense rt slot (1-padded)
         rt_b:   [RT] primitive index for each rt build (0-padded)
         grp_a:  [G] dense rt slot (0 = identity) per group (pad: 0)
         grp_b:  [G] rn code per group (pad: 1)
         grp_n:  [G] #positions per group (pad: 0)
         qC:     [npos] rn code for each position (emission order)
       G, RT: uniform (padded) group/rt-build counts, G even."""
    from concourse import bass, bacc, mybir

    f32 = mybir.dt.float32
    f32r = mybir.dt.float32r
    f16 = mybir.dt.float16
    Sub = mybir.AluOpType.subtract
    Add = mybir.AluOpType.add

    RT_STRIDE = (1 + RT) * ENT

    nc = bacc.Bacc("TRN2", target_bir_lowering=False, debug=debug)

    prims_ext = nc.dram_tensor("prims", [2, DIM, DIM], f32, kind="ExternalInput")
    ident_ext = nc.dram_tensor("ident", [DIM, DIM], f32, kind="ExternalInput")
    assert npos % 4 == 0
    nquad = npos // 4
    out_ext = nc.dram_tensor("out", [npos, P, 2 * DIM], f16, kind="ExternalOutput")

    with contextlib.ExitStack() as ctx:
        sem = {}
        for name in ["id_sem", "pr0_sem", "pr1_sem",
                     "pe_sem", "dve_sem", "act_sem",
                     "mm1_sem", "mm2_sem", "dvex_sem",
                     "oqa_sem", "oqd_sem",
                     "dma_q0", "dma_q1"]:
            sem[name] = ctx.enter_context(nc.semaphore(name))

        rn = ctx.enter_context(nc.sbuf_tensor("rn", [P, NAT_STRIDE], f16))
        rt = ctx.enter_context(nc.sbuf_tensor("rt", [P, RT_STRIDE], f16))
        pbf = ctx.enter_context(nc.sbuf_tensor("pbf", [P, 2, 2, DIM], f16))
        identf = ctx.enter_context(nc.sbuf_tensor("identf", [P, 2, DIM], f32))
        identr = ctx.enter_context(nc.sbuf_tensor("identr", [P, 2, DIM], f32r))
        i6 = ctx.enter_context(nc.sbuf_tensor("i6", [P, 2, DIM], f32r))
        i2 = ctx.enter_context(nc.sbuf_tensor("i2", [P, 2, DIM], f32r))
        prim = ctx.enter_context(nc.sbuf_tensor("prim", [P, 2, 2, DIM], f32))
        an_ = [ctx.enter_context(nc.sbuf_tensor(f"an{b}", [P, 2, DIM], f32r))
               for b in range(2)]
        a24 = [ctx.enter_context(nc.sbuf_tensor(f"a24_{b}", [P, 2, DIM], f32r))
               for b in range(2)]
        ye = [ctx.enter_context(nc.sbuf_tensor(f"ye{b}", [P, 2, DIM], f32r))
              for b in range(2)]
        yt = [ctx.enter_context(nc.sbuf_tensor(f"yt{b}", [P, 2, DIM], f32r))
              for b in range(2)]
        stag_x = ctx.enter_context(nc.sbuf_tensor("stag_x", [P, NSX, 2, DIM], f16))
        outb = ctx.enter_context(nc.sbuf_tensor("outb", [P, 8, 2, DIM], f16))
        scr = ctx.enter_context(nc.sbuf_tensor("scr", [P, 2 * P], f16))
        psA = ctx.enter_context(nc.psum_tensor("psA", [P, NSTAGE, 2, DIM], f32))
        psB = ctx.enter_context(nc.psum_tensor("psB", [P, NSTAGE, 2, DIM], f32))

        def bank(k):
            # 8-bank rotation across psA (0-3) and psB (4-7)
            return (psA if k < 4 else psB)[:, k % 4, :, :]

        ident128 = identf[:, 0, 0:P]

        def ent3(q):
            return bass.AP(rn, (q - 1) * ENT, [[NAT_STRIDE, P], [DIM, 2], [1, DIM]])

        def ent3t(s):
            return bass.AP(rt, s * ENT, [[RT_STRIDE, P], [DIM, 2], [1, DIM]])

        cnt = {k: 0 for k in sem}
        entry_done = {}
        pe_prog, dve_prog, act_prog, sync_prog, gps_prog = [], [], [], [], []

        # ---------------- DMA in ----------------
        def s_in(s):
            s.dma_start(identf[:, 0, :], ident_ext[0:P, :]).then_inc(sem["id_sem"], 16)
            s.dma_start(identf[:, 1, :], ident_ext[P:2 * P, :]).then_inc(sem["id_sem"], 16)
            for b in range(2):
                s.dma_start(prim[:, b, :, :],
                            bass.AP(prims_ext, b * DIM * DIM,
                                    [[DIM, P], [P * DIM, 2], [1, DIM]]),
                            ).then_inc(sem[f"pr{b}_sem"], 16)
        sync_prog.append(s_in)

        # ---------------- PE warmup (HAM) ----------------
        # scr is uninitialized SBUF: the results are garbage written to a
        # psum bank that is fully overwritten (start=True) before first use.
        def p_warm(t):
            for _ in range(WARMUP_MM):
                t.matmul(psB[:, 3, 0, :], scr[:, 0:P], scr[:, 0:2 * P],
                         start=True, stop=True)
        pe_prog.append(p_warm)

        # ---------------- ACT ident prep ----------------
        def a_ident(a):
            a.wait_ge(sem["id_sem"], 32)
            a.mul(identr[:, :, :], identf[:, :, :], 1.0)
            a.mul(i6[:, :, :], identf[:, :, :], 1.0 / 6.0)
            a.mul(i2[:, :, :], identf[:, :, :],
                  0.5).then_inc(sem["act_sem"], 1)
            # table identity entries (needed much later, by builds/mm1)
            a.mul(ent3(1), identf[:, :, :], 1.0)
            a.mul(ent3t(0), identf[:, :, :], 1.0).then_inc(sem["act_sem"], 1)
        act_prog.append(a_ident)
        cnt["act_sem"] += 2
        iprep_act = 1           # i6/i2/identr ready (act_sem)
        ident_act = 2           # table ident entries ready (act_sem)

        # ---------------- expm ----------------
        inv2s = 1.0 / (2.0 ** EXPM_S)

        for b in range(2):
            def p_tr(t, b=b):
                t.wait_ge(sem[f"pr{b}_sem"], 16)
                if b == 0:
                    t.wait_ge(sem["id_sem"], 32)
                last = None
                for kc in range(2):
                    for mc in range(2):
                        last = t.transpose(
                            out=bank(b)[:, kc, mc * P:(mc + 1) * P],
                            in_=prim[:, b, mc, kc * P:(kc + 1) * P],
                            identity=ident128)
                last.then_inc(sem["pe_sem"], 1)
            pe_prog.append(p_tr)
            cnt["pe_sem"] += 1

        prep_done = {}
        prep_act = {}
        for b in range(2):
            def d_prep(d, b=b, w=b + 1):
                d.wait_ge(sem["pe_sem"], w)
                d.tensor_tensor(out=ye[b][:, :, :], in0=bank(b)[:, :, :],
                                in1=prim[:, b, :, :], op=Sub)
                d.drain()
                d.tensor_scalar_mul(an_[b][:, :, :], ye[b][:, :, :], inv2s)
                d.tensor_scalar_mul(a24[b][:, :, :], ye[b][:, :, :],
                                    -inv2s / 24.0).then_inc(sem["dve_sem"], 1)
            dve_prog.append(d_prep)
            cnt["dve_sem"] += 1
            prep_done[b] = cnt["dve_sem"]
            prep_act[b] = iprep_act

        chains = [(0, 0), (0, 1), (1, 0), (1, 1)]
        ybuf = {(b, s): (ye[b] if s == 0 else yt[b])
                for b in range(2) for s in (0, 1)}
        bank_of = {c: 2 + i for i, c in enumerate(chains)}

        def emit_mm_fused(t, bk, parts, inc=None):
            last = None
            for mc in range(2):
                ops = [(lh, rh, kc) for lh, rh in parts for kc in range(2)]
                for idx, (lh, rh, kc) in enumerate(ops):
                    last = t.matmul(bank(bk)[:, mc, :],
                                    lh[:, kc, mc * P:(mc + 1) * P],
                                    rh[:, kc, :],
                                    start=(idx == 0),
                                    stop=(idx == len(ops) - 1))
            if inc is not None:
                last.then_inc(sem[inc], 1)
            return last

        copy_done = {}
        mm_done = {}
        addend = [i6, i2, identr]

        def wait_cds(t, *cds):
            best = {}
            for sname, c in cds:
                best[sname] = max(best.get(sname, 0), c)
            for sname, c in best.items():
                t.wait_ge(sem[sname], c)

        def emit_psum_copy(bk, d0, d1, full, add_ident=False, b=0):
            """Copy psum bank bk -> full; b==1 plain copies go to ACT."""
            w = cnt["pe_sem"]
            if add_ident or b == 0:
                def d_c(d, bk=bk, full=full, w=w, add_ident=add_ident):
                    d.wait_ge(sem["pe_sem"], w)
                    if add_ident:
                        d.tensor_tensor(out=full, in0=bank(bk)[:, :, :],
                                        in1=identf[:, :, :],
                                        op=Add).then_inc(sem["dve_sem"], 1)
                    else:
                        d.tensor_copy(full, bank(bk)[:, :, :],
                                      ).then_inc(sem["dve_sem"], 1)
                dve_prog.append(d_c)
                cnt["dve_sem"] += 1
                return (("dve_sem", cnt["dve_sem"]),)

            def a_c(a, bk=bk, full=full, w=w):
                a.wait_ge(sem["pe_sem"], w)
                a.mul(full, bank(bk)[:, :, :],
                      1.0).then_inc(sem["act_sem"], 1)
            act_prog.append(a_c)
            cnt["act_sem"] += 1
            return (("act_sem", cnt["act_sem"]),)

        for step in range(3):
            for b in range(2):
                s = 0
                wd = ((("dve_sem", prep_done[b]),) if step == 0
                      else copy_done[(b, s)])

                def p_h(t, b=b, s=s, step=step, wd=wd):
                    wait_cds(t, *wd)
                    if step == 0:
                        t.wait_ge(sem["act_sem"], prep_act[b])
                    bk = bank_of[(b, s)]
                    main = ((an_[b], a24[b]) if step == 0
                            else (an_[b], ybuf[(b, s)]))
                    emit_mm_fused(t, bk,
                                  [main, (an_[b], addend[step])],
                                  inc="pe_sem")
                pe_prog.append(p_h)
                cnt["pe_sem"] += 1
                mm_done[(b, s)] = cnt["pe_sem"]

                copy_done[(b, s)] = emit_psum_copy(
                    bank_of[(b, s)], None, None,
                    full=ybuf[(b, s)][:, :, :],
                    add_ident=(step == 2), b=b)

        # yt[b] = ye[b]^T via matmul against identity: T4(-x) == T4(x)^T
        for b in range(2):
            def p_tt(t, b=b, w=copy_done[(b, 0)]):
                wait_cds(t, *w)
                emit_mm_fused(t, bank_of[(b, 1)],
                              [(ybuf[(b, 0)], identr)], inc="pe_sem")
            pe_prog.append(p_tt)
            cnt["pe_sem"] += 1
            mm_done[(b, 1)] = cnt["pe_sem"]

            copy_done[(b, 1)] = emit_psum_copy(
                bank_of[(b, 1)], None, None,
                full=ybuf[(b, 1)][:, :, :], b=b)

        for sq in range(EXPM_S):
            last_sq = (sq == EXPM_S - 1)
            # last round keeps the s=1 chains too: their products are
            # P_b^T = mats[b], copied straight into table entries rn[2+b]
            active = list(chains)
            for (b, s) in active:
                def p_sq(t, b=b, s=s,
                         w0=copy_done[(b, 0)], w1=copy_done[(b, 1)]):
                    wait_cds(t, *w0, *w1)
                    emit_mm_fused(t, bank_of[(b, s)],
                                  [(ybuf[(b, 1 - s)], ybuf[(b, s)])],
                                  inc="pe_sem")
                pe_prog.append(p_sq)
                cnt["pe_sem"] += 1
                mm_done[(b, s)] = cnt["pe_sem"]

            for (b, s) in active:
                if last_sq:
                    full = pbf[:, b, :, :] if s == 0 else ent3(2 + b)
                else:
                    full = ybuf[(b, s)][:, :, :]
                # both of this b's products must be in psum before the
                # ybuf they read gets overwritten
                saved = cnt["pe_sem"]
                cnt["pe_sem"] = (mm_done[(b, s)] if last_sq
                                 else max(mm_done[(b, 0)], mm_done[(b, 1)]))
                copy_done[(b, s)] = emit_psum_copy(bank_of[(b, s)], None,
                                                   None, full=full, b=b)
                cnt["pe_sem"] = saved

        expm_cds = tuple(copy_done[(0, 0)]) + tuple(copy_done[(1, 0)])
        # P_b^T copies double as table entries rn[2], rn[3]
        ent_p = {2 + b: copy_done[(b, 1)][0] for b in range(2)}

        # ---------------- rn table build (uniform) ----------------
        # rn[q] for q=2..63; parent rn[q>>1], primitive pbf[q&1].
        pe_wt = _WaitTracker()
        dve_wt = _WaitTracker()
        act_wt = _WaitTracker()
        bank_owner = {}
        entry_done[1] = ("act_sem", ident_act)
        entry_done[2] = ent_p[2]
        entry_done[3] = ent_p[3]

        nb_total = 62 + RT
        for j in range(2, 62):
            q = j + 2
            b = q & 1
            par = q >> 1
            bk = j % 8

            waits = []
            if j < 8:
                waits.extend(expm_cds)
            waits.append(entry_done[par])
            if bk in bank_owner:
                waits.append(bank_owner[bk])

            def p_build(t, b=b, par=par, bk=bk, waits=tuple(waits)):
                for s_, c_ in waits:
                    pe_wt.wait(t, sem[s_], c_)
                last = None
                for mc in range(2):
                    for kc in range(2):
                        last = t.matmul(bank(bk)[:, mc, :],
                                        pbf[:, b, kc, mc * P:(mc + 1) * P],
                                        ent3(par)[:, kc, :],
                                        start=(kc == 0), stop=(kc == 1))
                last.then_inc(sem["pe_sem"], 1)
            pe_prog.append(p_build)
            cnt["pe_sem"] += 1

            ceng = "dve_sem" if j % 2 == 0 else "act_sem"
            prog = dve_prog if j % 2 == 0 else act_prog
            wtr = dve_wt if j % 2 == 0 else act_wt

            def x_copy(e, q=q, bk=bk, w=cnt["pe_sem"], ceng=ceng, wtr=wtr):
                wtr.wait(e, sem["pe_sem"], w)
                if ceng == "dve_sem":
                    e.tensor_copy(ent3(q), bank(bk)[:, :, :],
                                  ).then_inc(sem[ceng], 1)
                else:
                    e.mul(ent3(q), bank(bk)[:, :, :],
                          1.0).then_inc(sem[ceng], 1)
            prog.append(x_copy)
            cnt[ceng] += 1
            entry_done[q] = (ceng, cnt[ceng])
            bank_owner[bk] = (ceng, cnt[ceng])

        # ---------------- per-core section (PE only) ----------------
        # rt builds (copies are uniform, below) + position loop.
        pe_base = cnt["pe_sem"]
        rt_bank_owner = dict(bank_owner)

        def p_core(t):
            pid = t.partition_id()
            for c in t.Switch(pid, NCORES):
                cd = cores[c]
                wt = _WaitTracker()
                # inherit uniform guarantees
                wt.guar = dict(pe_wt.guar)

                # --- rt builds: rt slot 1+j = rn[par]^T-form product ---
                for j in range(RT):
                    par = int(cd["rt_par"][j])
                    b = int(cd["rt_b"][j])
                    bk = (62 + j) % 8
                    s_, c_ = entry_done[par]
                    wt.wait(t, sem[s_], c_)
                    s_, c_ = rt_bank_owner[bk]
                    wt.wait(t, sem[s_], c_)
                    last = None
                    for mc in range(2):
                        for kc in range(2):
                            last = t.matmul(bank(bk)[:, mc, :],
                                            ent3(par)[:, kc, mc * P:(mc + 1) * P],
                                            pbf[:, b, kc, :],
                                            start=(kc == 0), stop=(kc == 1))
                    last.then_inc(sem["pe_sem"], 1)

                # --- position loop (paced mm1/mm2 interleave) ---
                wt.wait(t, sem["dve_sem"], bd_total)
                wt.wait(t, sem["act_sem"], ba_total)
                g_of = []
                for g in range(G):
                    g_of += [g] * int(cd["grp_n"][g])
                assert len(g_of) == npos

                def emit_mm1(g, cd=cd, t=t, wt=wt):
                    wt.wait(t, sem["dvex_sem"], (g - 4) // 2 + 1)
                    slotA = int(cd["grp_a"][g])
                    qB = int(cd["grp_b"][g])
                    last = None
                    for mc in range(2):
                        for kc in range(2):
                            last = t.matmul(
                                psA[:, g % 4, mc, :],
                                ent3(qB)[:, kc, mc * P:(mc + 1) * P],
                                ent3t(slotA)[:, kc, :],
                                start=(kc == 0), stop=(kc == 1))
                    last.then_inc(sem["mm1_sem"], 1)

                def emit_mm2(p, cd=cd, t=t, wt=wt, g_of=g_of):
                    h = g_of[p]
                    wt.wait(t, sem["dvex_sem"], h // 2 + 1)
                    if p >= 4:
                        wt.wait(t, sem["oqa_sem"], (p - 4) // 2 + 1)
                    qC = int(cd["qC"][p])
                    last = None
                    for mc in range(2):
                        for kc in range(2):
                            last = t.matmul(
                                psB[:, p % 4, mc, :],
                                stag_x[:, h % NSX, kc, mc * P:(mc + 1) * P],
                                ent3(qC)[:, kc, :],
                                start=(kc == 0), stop=(kc == 1))
                    last.then_inc(sem["mm2_sem"], 1)

                p = 0
                for g in range(G):
                    # deadline: stag slot of group g-DEADLINE is recycled
                    # by the copy following mm1(g) -- drain its mm2s first
                    while p < npos and g_of[p] <= g - DEADLINE:
                        emit_mm2(p)
                        p += 1
                    emit_mm1(g)
                    target = ((g + 1) * npos) // G
                    while p < npos and p < target and g_of[p] <= g - LAG:
                        emit_mm2(p)
                        p += 1
                while p < npos:
                    emit_mm2(p)
                    p += 1
        pe_prog.append(p_core)

        # uniform rt copies (dense slots; psum banks rotate uniformly)
        for j in range(RT):
            bk = (62 + j) % 8
            ceng = "dve_sem" if (62 + j) % 2 == 0 else "act_sem"
            prog = dve_prog if (62 + j) % 2 == 0 else act_prog
            wtr = dve_wt if (62 + j) % 2 == 0 else act_wt

            def rt_copy(e, j=j, bk=bk, w=pe_base + j + 1, ceng=ceng, wtr=wtr):
                wtr.wait(e, sem["pe_sem"], w)
                if ceng == "dve_sem":
                    e.tensor_copy(ent3t(1 + j), bank(bk)[:, :, :],
                                  ).then_inc(sem[ceng], 1)
                else:
                    e.mul(ent3t(1 + j), bank(bk)[:, :, :],
                          1.0).then_inc(sem[ceng], 1)
            prog.append(rt_copy)
            cnt[ceng] += 1
            bank_owner[bk] = (ceng, cnt[ceng])

        bd_total = cnt["dve_sem"]
        ba_total = cnt["act_sem"]

        # ---- ACT: stag pair copies (uniform) ----
        def a_pos(a):
            for j in range(G // 2):
                act_wt.wait(a, sem["mm1_sem"], 2 * j + 2)
                sl = (2 * j) % NSX
                bk = (2 * j) % 4
                a.mul(stag_x[:, sl:sl + 2, :, :],
                      psA[:, bk:bk + 2, :, :],
                      1.0).then_inc(sem["dvex_sem"], 1)
        act_prog.append(a_pos)

        # ---- DVE: output pair copies (uniform) ----
        # pair k = positions (2k, 2k+1): psB banks (2k%4, +1) -> outb
        # slots ((k%4)*2, +1); pair-granular DMA frees outb slots.
        def d_pos(d):
            for k in range(npos // 2):
                dve_wt.wait(d, sem["mm2_sem"], 2 * k + 2)
                if k >= 4:
                    dve_wt.wait(d, sem[f"dma_q{k % 2}"],
                                16 * ((k - 4) // 2 + 1))
                sl = (k % 4) * 2
                bk = (2 * k) % 4
                d.tensor_copy(outb[:, sl:sl + 2, :, :],
                              psB[:, bk:bk + 2, :, :],
                              ).then_inc(sem["oqa_sem"], 1)
        dve_prog.append(d_pos)

        # ---------------- Sync: output pair DMAs ----------------
        def s_pos(s):
            for k in range(npos // 2):
                s.wait_ge(sem["oqa_sem"], k + 1)
                sl = (k % 4) * 2
                dst = bass.AP(out_ext, 2 * k * P * 2 * DIM,
                              [[2 * DIM, P], [P * 2 * DIM, 2], [1, 2 * DIM]])
                s.dma_start(dst, outb[:, sl:sl + 2, :, :],
                            ).then_inc(sem[f"dma_q{k % 2}"], 16)
            s.wait_ge(sem["dma_q0"], 16 * (npos // 4))
            s.wait_ge(sem["dma_q1"], 16 * (npos // 4))
        sync_prog.append(s_pos)

        # ---------------- emit ----------------
        with nc.Block(no_gpsimd_drain=True) as block:
            @block.tensor
            def _(tensor):
                for fn in pe_prog:
                    fn(tensor)

            @block.vector
            def _(vector):
                for fn in dve_prog:
                    fn(vector)

            @block.scalar
            def _(scalar):
                for fn in act_prog:
                    fn(scalar)

            @block.sync
            def _(sync):
                for fn in sync_prog:
                    fn(sync)

            if gps_prog:
                @block.gpsimd
                def _(gpsimd):
                    for fn in gps_prog:
                        fn(gpsimd)

    return nc


def _host_indices(u):
    """u: (n,) int64 positions -> (idxA, idxB, idxC) int arrays."""
    u = u.astype(np.int64)
    blen = np.zeros_like(u)
    t = u.copy()
    while np.any(t > 0):
        blen = np.where(t > 0, blen + 1, blen)
        t >>= 1
    k = blen - 1
    tA = np.minimum(k, 6)
    idxA = (1 << tA) + (u & ((1 << tA) - 1))
    tB = np.clip(k - 6, 0, 5)
    idxB = (1 << tB) + ((u >> 6) & ((1 << tB) - 1))
    tC = np.clip(k - 11, 0, 5)
    idxC = (1 << tC) + ((u >> 11) & ((1 << tC) - 1))
    short = u < 64
    idxA = np.where(short, 1, idxA)
    idxB = np.where(short, u, idxB)
    assert idxA.max() < 128 and idxB.max() < 64 and idxC.max() < 64
    assert np.all((idxA == 1) | (idxA >= 64))
    return idxA, idxB, idxC


def _pack(u, npos):
    """Sort by (A,B) key, shard contiguously, per-core group structure."""
    n = len(u)
    idxA, idxB, idxC = _host_indices(u)
    key = idxA.astype(np.int64) * 64 + idxB
    order = np.argsort(key, kind="stable")

    cores = []
    G_list, RT_list = [], []
    perm = np.empty(n, np.int64)
    for c in range(NCORES):
        sl = slice(c * npos, (c + 1) * npos)
        o = order[sl]
        kk = key[o]
        qA, qB, qC = idxA[o], idxB[o], idxC[o]
        newg = np.ones(npos, bool)
        newg[1:] = kk[1:] != kk[:-1]
        starts = np.flatnonzero(newg)
        sizes = np.diff(np.append(starts, npos))
        # dense rt slots for this core's A codes (code 1 -> slot 0)
        acodes = np.unique(qA[starts])
        acodes = acodes[acodes > 1]
        slot_of = {1: 0}
        for j, q in enumerate(acodes):
            slot_of[int(q)] = 1 + j
        cores.append({
            "rt_codes": acodes,
            "grp_a": np.array([slot_of[int(q)] for q in qA[starts]], np.int64),
            "grp_b": qB[starts].astype(np.int64),
            "grp_n": sizes.astype(np.int64),
            "qC": qC.astype(np.int64),
        })
        G_list.append(len(starts))
        RT_list.append(len(acodes))
        perm[c * npos:(c + 1) * npos] = o

    G = max(G_list)
    G += G % 2  # even
    RT = max(RT_list)
    for cd in cores:
        g = len(cd["grp_n"])
        cd["grp_a"] = np.concatenate([cd["grp_a"], np.zeros(G - g, np.int64)])
        cd["grp_b"] = np.concatenate([cd["grp_b"], np.ones(G - g, np.int64)])
        cd["grp_n"] = np.concatenate([cd["grp_n"], np.zeros(G - g, np.int64)])
        r = len(cd["rt_codes"])
        par = np.ones(RT, np.int64)
        bb = np.zeros(RT, np.int64)
        par[:r] = cd["rt_codes"] >> 1
        bb[:r] = cd["rt_codes"] & 1
        cd["rt_par"] = par
        cd["rt_b"] = bb
    return cores, G, RT, perm


def kernel(primitives, identity, unique):
    global LAST_RESULTS
    from concourse.bass_utils import run_bass_kernel_spmd

    prims = np.ascontiguousarray(np.asarray(primitives, dtype=np.float32))
    u = np.asarray(unique).astype(np.int64).ravel()
    n = u.shape[0]
    assert n % NCORES == 0
    npos = n // NCORES

    cores, G, RT, perm = _pack(u, npos)
    eye = np.eye(DIM, dtype=np.float32)

    ckey = (npos, u.tobytes())
    if ckey not in _NC_CACHE:
        nc = _build_nc(npos, cores, G, RT)
        nc.compile()
        _NC_CACHE.clear()
        _NC_CACHE[ckey] = nc
    nc = _NC_CACHE[ckey]

    in_maps = [{"prims": prims, "ident": eye} for _ in range(NCORES)]

    import os
    trace_dir = os.environ.get("KERNEL_TRACE_DIR")
    res = run_bass_kernel_spmd(nc, in_maps, core_ids=list(range(NCORES)),
                               tmpdir=trace_dir)
    LAST_RESULTS = res

    parts = []
    for c in range(NCORES):
        o = np.asarray(res.results[c]["out"])
        o = o.reshape(npos, P, 2, DIM).transpose(0, 2, 1, 3)
        parts.append(o.reshape(npos, DIM, DIM).astype(np.float32))
    out = np.empty((n, DIM, DIM), np.float32)
    out[perm] = np.concatenate(parts, axis=0)

    ident = np.asarray(identity, dtype=np.float32)[0]
    if not np.allclose(ident, np.eye(DIM, dtype=np.float32)):
        out = np.einsum("ij,njk->nik", ident, out).astype(np.float32)
    return out


# revision 60
# speedup vs baseline: 1.1990x; 1.1990x over previous
"""Trainium2 Bass kernel for nn_BinaryPathEncoder — v2.

Math: 6+5+5 bit-chunk table decomposition.  maps[u] = A(u&63) @ B @ C where
A is the low-6-bit chunk (transposed table rt), B bits 6-10, C bits 11-15
(shared table rn).  Per position: mm1 = (A@B)^T (deduped over distinct
(A,B) keys, arbitrary group sizes), mm2 = out = (A@B) @ C.

v2 over baseline:
 - mm1 dedup with arbitrary group sizes (G ~= 163/core vs 190)
 - rt table per-core dense slots: only ~10 of 64 entries built per core
   (positions are sorted by (A,B) key so each core sees a narrow A band)
 - PE warmup matmuls during input-DMA head (HAM warm before expm)
 - expm scaling-squaring s=2 (one fewer squaring round)
 - transposes wait on DMA sem directly (not the DVE ident chain)
 - output copies 4-positions wide on ACT (amortize fixed overhead),
   output DMA per quad; redundant semaphore waits elided
"""

import contextlib
import numpy as np

DIM = 256
NCORES = 8
P = 128

ENT = 512          # f16 elems per table entry per partition (2*DIM)
NAT_E = 63         # rn: codes 1..63 (5-bit variable chunk products)
NAT_STRIDE = NAT_E * ENT

NSTAGE = 4         # mm1 psum banks (psA)
NSX = 8            # stag slots
LAG = 4            # min groups of mm1 lead before a group's mm2s start
DEADLINE = 8       # group h's mm2s must be emitted before mm1(h+DEADLINE)
EXPM_S = 2
EXPM_N = 4
WARMUP_MM = 20

_NC_CACHE = {}
LAST_RESULTS = None


class _WaitTracker:
    """Elide semaphore waits whose threshold is already guaranteed."""

    def __init__(self):
        self.guar = {}

    def wait(self, eng, sem, thr):
        if thr <= 0:
            return
        if self.guar.get(id(sem), -1) >= thr:
            return
        eng.wait_ge(sem, thr)
        self.guar[id(sem)] = thr


def _build_nc(npos, cores, G, RT, debug=False):
    """cores: per-core dict with keys:
         rt_par: [RT] parent rn code for each dense rt slot (1-padded)
         rt_b:   [RT] primitive index for each rt build (0-padded)
         grp_a:  [G] dense rt slot (0 = identity) per group (pad: 0)
         grp_b:  [G] rn code per group (pad: 1)
         grp_n:  [G] #positions per group (pad: 0)
         qC:     [npos] rn code for each position (emission order)
       G, RT: uniform (padded) group/rt-build counts, G even."""
    from concourse import bass, bacc, mybir

    f32 = mybir.dt.float32
    f32r = mybir.dt.float32r
    f16 = mybir.dt.float16
    Sub = mybir.AluOpType.subtract
    Add = mybir.AluOpType.add

    RT_STRIDE = (1 + RT) * ENT

    nc = bacc.Bacc("TRN2", target_bir_lowering=False, debug=debug)

    prims_ext = nc.dram_tensor("prims", [2, DIM, DIM], f32, kind="ExternalInput")
    ident_ext = nc.dram_tensor("ident", [DIM, DIM], f32, kind="ExternalInput")
    assert npos % 4 == 0
    nquad = npos // 4
    out_ext = nc.dram_tensor("out", [npos, P, 2 * DIM], f16, kind="ExternalOutput")

    with contextlib.ExitStack() as ctx:
        sem = {}
        for name in ["id_sem", "pr0_sem", "pr1_sem",
                     "pe_sem", "dve_sem", "act_sem",
                     "mm1_sem", "mm2_sem", "dvex_sem",
                     "oqa_sem", "oqd_sem",
                     "dma_q0", "dma_q1"]:
            sem[name] = ctx.enter_context(nc.semaphore(name))

        rn = ctx.enter_context(nc.sbuf_tensor("rn", [P, NAT_STRIDE], f16))
        rt = ctx.enter_context(nc.sbuf_tensor("rt", [P, RT_STRIDE], f16))
        pbf = ctx.enter_context(nc.sbuf_tensor("pbf", [P, 2, 2, DIM], f16))
        identf = ctx.enter_context(nc.sbuf_tensor("identf", [P, 2, DIM], f32))
        identr = ctx.enter_context(nc.sbuf_tensor("identr", [P, 2, DIM], f32r))
        i6 = ctx.enter_context(nc.sbuf_tensor("i6", [P, 2, DIM], f32r))
        i2 = ctx.enter_context(nc.sbuf_tensor("i2", [P, 2, DIM], f32r))
        prim = ctx.enter_context(nc.sbuf_tensor("prim", [P, 2, 2, DIM], f32))
        an_ = [ctx.enter_context(nc.sbuf_tensor(f"an{b}", [P, 2, DIM], f32r))
               for b in range(2)]
        a24 = [ctx.enter_context(nc.sbuf_tensor(f"a24_{b}", [P, 2, DIM], f32r))
               for b in range(2)]
        ye = [ctx.enter_context(nc.sbuf_tensor(f"ye{b}", [P, 2, DIM], f32r))
              for b in range(2)]
        yt = [ctx.enter_context(nc.sbuf_tensor(f"yt{b}", [P, 2, DIM], f32r))
              for b in range(2)]
        stag_x = ctx.enter_context(nc.sbuf_tensor("stag_x", [P, NSX, 2, DIM], f16))
        outb = ctx.enter_context(nc.sbuf_tensor("outb", [P, 8, 2, DIM], f16))
        scr = ctx.enter_context(nc.sbuf_tensor("scr", [P, 2 * P], f16))
        psA = ctx.enter_context(nc.psum_tensor("psA", [P, NSTAGE, 2, DIM], f32))
        psB = ctx.enter_context(nc.psum_tensor("psB", [P, NSTAGE, 2, DIM], f32))

        def bank(k):
            # 8-bank rotation across psA (0-3) and psB (4-7)
            return (psA if k < 4 else psB)[:, k % 4, :, :]

        ident128 = identf[:, 0, 0:P]

        def ent3(q):
            return bass.AP(rn, (q - 1) * ENT, [[NAT_STRIDE, P], [DIM, 2], [1, DIM]])

        def ent3t(s):
            return bass.AP(rt, s * ENT, [[RT_STRIDE, P], [DIM, 2], [1, DIM]])

        cnt = {k: 0 for k in sem}
        entry_done = {}
        pe_prog, dve_prog, act_prog, sync_prog, gps_prog = [], [], [], [], []

        # ---------------- DMA in ----------------
        def s_in(s):
            s.dma_start(identf[:, 0, :], ident_ext[0:P, :]).then_inc(sem["id_sem"], 16)
            s.dma_start(identf[:, 1, :], ident_ext[P:2 * P, :]).then_inc(sem["id_sem"], 16)
            for b in range(2):
                s.dma_start(prim[:, b, :, :],
                            bass.AP(prims_ext, b * DIM * DIM,
                                    [[DIM, P], [P * DIM, 2], [1, DIM]]),
                            ).then_inc(sem[f"pr{b}_sem"], 16)
        sync_prog.append(s_in)

        # ---------------- PE warmup (HAM) ----------------
        # scr is uninitialized SBUF: the results are garbage written to a
        # psum bank that is fully overwritten (start=True) before first use.
        def p_warm(t):
            for _ in range(WARMUP_MM):
                t.matmul(psB[:, 3, 0, :], scr[:, 0:P], scr[:, 0:2 * P],
                         start=True, stop=True)
        pe_prog.append(p_warm)

        # ---------------- ACT ident prep ----------------
        def a_ident(a):
            a.wait_ge(sem["id_sem"], 32)
            a.mul(identr[:, :, :], identf[:, :, :], 1.0)
            a.mul(i6[:, :, :], identf[:, :, :], 1.0 / 6.0)
            a.mul(i2[:, :, :], identf[:, :, :],
                  0.5).then_inc(sem["act_sem"], 1)
            # table identity entries (needed much later, by builds/mm1)
            a.mul(ent3(1), identf[:, :, :], 1.0)
            a.mul(ent3t(0), identf[:, :, :], 1.0).then_inc(sem["act_sem"], 1)
        act_prog.append(a_ident)
        cnt["act_sem"] += 2
        iprep_act = 1           # i6/i2/identr ready (act_sem)
        ident_act = 2           # table ident entries ready (act_sem)

        # ---------------- expm ----------------
        inv2s = 1.0 / (2.0 ** EXPM_S)

        for b in range(2):
            def p_tr(t, b=b):
                t.wait_ge(sem[f"pr{b}_sem"], 16)
                if b == 0:
                    t.wait_ge(sem["id_sem"], 32)
                last = None
                for kc in range(2):
                    for mc in range(2):
                        last = t.transpose(
                            out=bank(b)[:, kc, mc * P:(mc + 1) * P],
                            in_=prim[:, b, mc, kc * P:(kc + 1) * P],
                            identity=ident128)
                last.then_inc(sem["pe_sem"], 1)
            pe_prog.append(p_tr)
            cnt["pe_sem"] += 1

        prep_done = {}
        prep_act = {}
        for b in range(2):
            def d_prep(d, b=b, w=b + 1):
                d.wait_ge(sem["pe_sem"], w)
                d.tensor_tensor(out=ye[b][:, :, :], in0=bank(b)[:, :, :],
                                in1=prim[:, b, :, :], op=Sub)
                d.drain()
                d.tensor_scalar_mul(an_[b][:, :, :], ye[b][:, :, :], inv2s)
                d.tensor_scalar_mul(a24[b][:, :, :], ye[b][:, :, :],
                                    -inv2s / 24.0).then_inc(sem["dve_sem"], 1)
            dve_prog.append(d_prep)
            cnt["dve_sem"] += 1
            prep_done[b] = cnt["dve_sem"]
            prep_act[b] = iprep_act

        chains = [(0, 0), (0, 1), (1, 0), (1, 1)]
        ybuf = {(b, s): (ye[b] if s == 0 else yt[b])
                for b in range(2) for s in (0, 1)}
        bank_of = {c: 2 + i for i, c in enumerate(chains)}

        def emit_mm_fused(t, bk, parts, inc=None):
            last = None
            for mc in range(2):
                ops = [(lh, rh, kc) for lh, rh in parts for kc in range(2)]
                for idx, (lh, rh, kc) in enumerate(ops):
                    last = t.matmul(bank(bk)[:, mc, :],
                                    lh[:, kc, mc * P:(mc + 1) * P],
                                    rh[:, kc, :],
                                    start=(idx == 0),
                                    stop=(idx == len(ops) - 1))
            if inc is not None:
                last.then_inc(sem[inc], 1)
            return last

        copy_done = {}
        mm_done = {}
        addend = [i6, i2, identr]

        def wait_cds(t, *cds):
            best = {}
            for sname, c in cds:
                best[sname] = max(best.get(sname, 0), c)
            for sname, c in best.items():
                t.wait_ge(sem[sname], c)

        def emit_psum_copy(bk, d0, d1, full, add_ident=False, b=0):
            """Copy psum bank bk -> full; b==1 plain copies go to ACT."""
            w = cnt["pe_sem"]
            if add_ident or b == 0:
                def d_c(d, bk=bk, full=full, w=w, add_ident=add_ident):
                    d.wait_ge(sem["pe_sem"], w)
                    if add_ident:
                        d.tensor_tensor(out=full, in0=bank(bk)[:, :, :],
                                        in1=identf[:, :, :],
                                        op=Add).then_inc(sem["dve_sem"], 1)
                    else:
                        d.tensor_copy(full, bank(bk)[:, :, :],
                                      ).then_inc(sem["dve_sem"], 1)
                dve_prog.append(d_c)
                cnt["dve_sem"] += 1
                return (("dve_sem", cnt["dve_sem"]),)

            def a_c(a, bk=bk, full=full, w=w):
                a.wait_ge(sem["pe_sem"], w)
                a.mul(full, bank(bk)[:, :, :],
                      1.0).then_inc(sem["act_sem"], 1)
            act_prog.append(a_c)
            cnt["act_sem"] += 1
            return (("act_sem", cnt["act_sem"]),)

        for step in range(3):
            for b in range(2):
                s = 0
                wd = ((("dve_sem", prep_done[b]),) if step == 0
                      else copy_done[(b, s)])

                def p_h(t, b=b, s=s, step=step, wd=wd):
                    wait_cds(t, *wd)
                    if step == 0:
                        t.wait_ge(sem["act_sem"], prep_act[b])
                    bk = bank_of[(b, s)]
                    main = ((an_[b], a24[b]) if step == 0
                            else (an_[b], ybuf[(b, s)]))
                    emit_mm_fused(t, bk,
                                  [main, (an_[b], addend[step])],
                                  inc="pe_sem")
                pe_prog.append(p_h)
                cnt["pe_sem"] += 1
                mm_done[(b, s)] = cnt["pe_sem"]

                copy_done[(b, s)] = emit_psum_copy(
                    bank_of[(b, s)], None, None,
                    full=ybuf[(b, s)][:, :, :],
                    add_ident=(step == 2), b=b)

        # yt[b] = ye[b]^T via matmul against identity: T4(-x) == T4(x)^T
        for b in range(2):
            def p_tt(t, b=b, w=copy_done[(b, 0)]):
                wait_cds(t, *w)
                emit_mm_fused(t, bank_of[(b, 1)],
                              [(ybuf[(b, 0)], identr)], inc="pe_sem")
            pe_prog.append(p_tt)
            cnt["pe_sem"] += 1
            mm_done[(b, 1)] = cnt["pe_sem"]

            copy_done[(b, 1)] = emit_psum_copy(
                bank_of[(b, 1)], None, None,
                full=ybuf[(b, 1)][:, :, :], b=b)

        for sq in range(EXPM_S):
            last_sq = (sq == EXPM_S - 1)
            # last round keeps the s=1 chains too: their products are
            # P_b^T = mats[b], copied straight into table entries rn[2+b]
            active = list(chains)
            for (b, s) in active:
                def p_sq(t, b=b, s=s,
                         w0=copy_done[(b, 0)], w1=copy_done[(b, 1)]):
                    wait_cds(t, *w0, *w1)
                    emit_mm_fused(t, bank_of[(b, s)],
                                  [(ybuf[(b, 1 - s)], ybuf[(b, s)])],
                                  inc="pe_sem")
                pe_prog.append(p_sq)
                cnt["pe_sem"] += 1
                mm_done[(b, s)] = cnt["pe_sem"]

            for (b, s) in active:
                if last_sq:
                    full = pbf[:, b, :, :] if s == 0 else ent3(2 + b)
                else:
                    full = ybuf[(b, s)][:, :, :]
                # both of this b's products must be in psum before the
                # ybuf they read gets overwritten
                saved = cnt["pe_sem"]
                cnt["pe_sem"] = (mm_done[(b, s)] if last_sq
                                 else max(mm_done[(b, 0)], mm_done[(b, 1)]))
                copy_done[(b, s)] = emit_psum_copy(bank_of[(b, s)], None,
                                                   None, full=full, b=b)
                cnt["pe_sem"] = saved

        expm_cds = tuple(copy_done[(0, 0)]) + tuple(copy_done[(1, 0)])
        # P_b^T copies double as table entries rn[2], rn[3]
        ent_p = {2 + b: copy_done[(b, 1)][0] for b in range(2)}

        # ---------------- rn table build (uniform) ----------------
        # rn[q] for q=2..63; parent rn[q>>1], primitive pbf[q&1].
        pe_wt = _WaitTracker()
        dve_wt = _WaitTracker()
        act_wt = _WaitTracker()
        bank_owner = {}
        entry_done[1] = ("act_sem", ident_act)
        entry_done[2] = ent_p[2]
        entry_done[3] = ent_p[3]

        nb_total = 62 + RT
        for j in range(2, 62):
            q = j + 2
            b = q & 1
            par = q >> 1
            bk = j % 8

            waits = []
            if j < 8:
                waits.extend(expm_cds)
            waits.append(entry_done[par])
            if bk in bank_owner:
                waits.append(bank_owner[bk])

            def p_build(t, b=b, par=par, bk=bk, waits=tuple(waits)):
                for s_, c_ in waits:
                    pe_wt.wait(t, sem[s_], c_)
                last = None
                for mc in range(2):
                    for kc in range(2):
                        last = t.matmul(bank(bk)[:, mc, :],
                                        pbf[:, b, kc, mc * P:(mc + 1) * P],
                                        ent3(par)[:, kc, :],
                                        start=(kc == 0), stop=(kc == 1))
                last.then_inc(sem["pe_sem"], 1)
            pe_prog.append(p_build)
            cnt["pe_sem"] += 1

            ceng = "dve_sem" if j % 2 == 0 else "act_sem"
            prog = dve_prog if j % 2 == 0 else act_prog
            wtr = dve_wt if j % 2 == 0 else act_wt

            def x_copy(e, q=q, bk=bk, w=cnt["pe_sem"], ceng=ceng, wtr=wtr):
                wtr.wait(e, sem["pe_sem"], w)
                if ceng == "dve_sem":
                    e.tensor_copy(ent3(q), bank(bk)[:, :, :],
                                  ).then_inc(sem[ceng], 1)
                else:
                    e.mul(ent3(q), bank(bk)[:, :, :],
                          1.0).then_inc(sem[ceng], 1)
            prog.append(x_copy)
            cnt[ceng] += 1
            entry_done[q] = (ceng, cnt[ceng])
            bank_owner[bk] = (ceng, cnt[ceng])

        # ---------------- per-core section (PE only) ----------------
        # rt builds (copies are uniform, below) + position loop.
        pe_base = cnt["pe_sem"]
        rt_bank_owner = dict(bank_owner)

        def p_core(t):
            pid = t.partition_id()
            for c in t.Switch(pid, NCORES):
                cd = cores[c]
                wt = _WaitTracker()
                # inherit uniform guarantees
                wt.guar = dict(pe_wt.guar)

                # --- rt builds: rt slot 1+j = rn[par]^T-form product ---
                for j in range(RT):
                    par = int(cd["rt_par"][j])
                    b = int(cd["rt_b"][j])
                    bk = (62 + j) % 8
                    s_, c_ = entry_done[par]
                    wt.wait(t, sem[s_], c_)
                    s_, c_ = rt_bank_owner[bk]
                    wt.wait(t, sem[s_], c_)
                    last = None
                    for mc in range(2):
                        for kc in range(2):
                            last = t.matmul(bank(bk)[:, mc, :],
                                            ent3(par)[:, kc, mc * P:(mc + 1) * P],
                                            pbf[:, b, kc, :],
                                            start=(kc == 0), stop=(kc == 1))
                    last.then_inc(sem["pe_sem"], 1)

                # --- position loop (paced mm1/mm2 interleave) ---
                wt.wait(t, sem["dve_sem"], bd_total)
                wt.wait(t, sem["act_sem"], ba_total)
                g_of = []
                for g in range(G):
                    g_of += [g] * int(cd["grp_n"][g])
                assert len(g_of) == npos

                def emit_mm1(g, cd=cd, t=t, wt=wt):
                    wt.wait(t, sem["dvex_sem"], (g - 4) // 2 + 1)
                    slotA = int(cd["grp_a"][g])
                    qB = int(cd["grp_b"][g])
                    last = None
                    for mc in range(2):
                        for kc in range(2):
                            last = t.matmul(
                                psA[:, g % 4, mc, :],
                                ent3(qB)[:, kc, mc * P:(mc + 1) * P],
                                ent3t(slotA)[:, kc, :],
                                start=(kc == 0), stop=(kc == 1))
                    last.then_inc(sem["mm1_sem"], 1)

                def emit_mm2(p, cd=cd, t=t, wt=wt, g_of=g_of):
                    h = g_of[p]
                    wt.wait(t, sem["dvex_sem"], h // 2 + 1)
                    if p >= 4:
                        wt.wait(t, sem["oqa_sem"], (p - 4) // 2 + 1)
                    qC = int(cd["qC"][p])
                    last = None
                    for mc in range(2):
                        for kc in range(2):
                            last = t.matmul(
                                psB[:, p % 4, mc, :],
                                stag_x[:, h % NSX, kc, mc * P:(mc + 1) * P],
                                ent3(qC)[:, kc, :],
                                start=(kc == 0), stop=(kc == 1))
                    last.then_inc(sem["mm2_sem"], 1)

                p = 0
                for g in range(G):
                    # deadline: stag slot of group g-DEADLINE is recycled
                    # by the copy following mm1(g) -- drain its mm2s first
                    while p < npos and g_of[p] <= g - DEADLINE:
                        emit_mm2(p)
                        p += 1
                    emit_mm1(g)
                    target = ((g + 1) * npos) // G
                    burst = 0
                    while (p < npos and p < target and g_of[p] <= g - LAG
                           and burst < 3):
                        emit_mm2(p)
                        p += 1
                        burst += 1
                while p < npos:
                    emit_mm2(p)
                    p += 1
        pe_prog.append(p_core)

        # uniform rt copies (dense slots; psum banks rotate uniformly)
        for j in range(RT):
            bk = (62 + j) % 8
            ceng = "dve_sem" if (62 + j) % 2 == 0 else "act_sem"
            prog = dve_prog if (62 + j) % 2 == 0 else act_prog
            wtr = dve_wt if (62 + j) % 2 == 0 else act_wt

            def rt_copy(e, j=j, bk=bk, w=pe_base + j + 1, ceng=ceng, wtr=wtr):
                wtr.wait(e, sem["pe_sem"], w)
                if ceng == "dve_sem":
                    e.tensor_copy(ent3t(1 + j), bank(bk)[:, :, :],
                                  ).then_inc(sem[ceng], 1)
                else:
                    e.mul(ent3t(1 + j), bank(bk)[:, :, :],
                          1.0).then_inc(sem[ceng], 1)
            prog.append(rt_copy)
            cnt[ceng] += 1
            bank_owner[bk] = (ceng, cnt[ceng])

        bd_total = cnt["dve_sem"]
        ba_total = cnt["act_sem"]

        # ---- ACT: stag pair copies (uniform) ----
        def a_pos(a):
            for j in range(G // 2):
                act_wt.wait(a, sem["mm1_sem"], 2 * j + 2)
                sl = (2 * j) % NSX
                bk = (2 * j) % 4
                a.mul(stag_x[:, sl:sl + 2, :, :],
                      psA[:, bk:bk + 2, :, :],
                      1.0).then_inc(sem["dvex_sem"], 1)
        act_prog.append(a_pos)

        # ---- DVE: output pair copies (uniform) ----
        # pair k = positions (2k, 2k+1): psB banks (2k%4, +1) -> outb
        # slots ((k%4)*2, +1); pair-granular DMA frees outb slots.
        def d_pos(d):
            for k in range(npos // 2):
                dve_wt.wait(d, sem["mm2_sem"], 2 * k + 2)
                if k >= 4:
                    dve_wt.wait(d, sem[f"dma_q{k % 2}"],
                                16 * ((k - 4) // 2 + 1))
                sl = (k % 4) * 2
                bk = (2 * k) % 4
                d.tensor_copy(outb[:, sl:sl + 2, :, :],
                              psB[:, bk:bk + 2, :, :],
                              ).then_inc(sem["oqa_sem"], 1)
        dve_prog.append(d_pos)

        # ---------------- Sync: output pair DMAs ----------------
        def s_pos(s):
            for k in range(npos // 2):
                s.wait_ge(sem["oqa_sem"], k + 1)
                sl = (k % 4) * 2
                dst = bass.AP(out_ext, 2 * k * P * 2 * DIM,
                              [[2 * DIM, P], [P * 2 * DIM, 2], [1, 2 * DIM]])
                s.dma_start(dst, outb[:, sl:sl + 2, :, :],
                            ).then_inc(sem[f"dma_q{k % 2}"], 16)
            s.wait_ge(sem["dma_q0"], 16 * (npos // 4))
            s.wait_ge(sem["dma_q1"], 16 * (npos // 4))
        sync_prog.append(s_pos)

        # ---------------- emit ----------------
        with nc.Block(no_gpsimd_drain=True) as block:
            @block.tensor
            def _(tensor):
                for fn in pe_prog:
                    fn(tensor)

            @block.vector
            def _(vector):
                for fn in dve_prog:
                    fn(vector)

            @block.scalar
            def _(scalar):
                for fn in act_prog:
                    fn(scalar)

            @block.sync
            def _(sync):
                for fn in sync_prog:
                    fn(sync)

            if gps_prog:
                @block.gpsimd
                def _(gpsimd):
                    for fn in gps_prog:
                        fn(gpsimd)

    return nc


def _host_indices(u):
    """u: (n,) int64 positions -> (idxA, idxB, idxC) int arrays."""
    u = u.astype(np.int64)
    blen = np.zeros_like(u)
    t = u.copy()
    while np.any(t > 0):
        blen = np.where(t > 0, blen + 1, blen)
        t >>= 1
    k = blen - 1
    tA = np.minimum(k, 6)
    idxA = (1 << tA) + (u & ((1 << tA) - 1))
    tB = np.clip(k - 6, 0, 5)
    idxB = (1 << tB) + ((u >> 6) & ((1 << tB) - 1))
    tC = np.clip(k - 11, 0, 5)
    idxC = (1 << tC) + ((u >> 11) & ((1 << tC) - 1))
    short = u < 64
    idxA = np.where(short, 1, idxA)
    idxB = np.where(short, u, idxB)
    assert idxA.max() < 128 and idxB.max() < 64 and idxC.max() < 64
    assert np.all((idxA == 1) | (idxA >= 64))
    return idxA, idxB, idxC


def _pack(u, npos):
    """Sort by (A,B) key, shard contiguously, per-core group structure."""
    n = len(u)
    idxA, idxB, idxC = _host_indices(u)
    key = idxA.astype(np.int64) * 64 + idxB
    order = np.argsort(key, kind="stable")

    cores = []
    G_list, RT_list = [], []
    perm = np.empty(n, np.int64)
    for c in range(NCORES):
        sl = slice(c * npos, (c + 1) * npos)
        o = order[sl]
        kk = key[o]
        qA, qB, qC = idxA[o], idxB[o], idxC[o]
        newg = np.ones(npos, bool)
        newg[1:] = kk[1:] != kk[:-1]
        starts = np.flatnonzero(newg)
        sizes = np.diff(np.append(starts, npos))
        # dense rt slots for this core's A codes (code 1 -> slot 0)
        acodes = np.unique(qA[starts])
        acodes = acodes[acodes > 1]
        slot_of = {1: 0}
        for j, q in enumerate(acodes):
            slot_of[int(q)] = 1 + j
        cores.append({
            "rt_codes": acodes,
            "grp_a": np.array([slot_of[int(q)] for q in qA[starts]], np.int64),
            "grp_b": qB[starts].astype(np.int64),
            "grp_n": sizes.astype(np.int64),
            "qC": qC.astype(np.int64),
        })
        G_list.append(len(starts))
        RT_list.append(len(acodes))
        perm[c * npos:(c + 1) * npos] = o

    G = max(G_list)
    G += G % 2  # even
    RT = max(RT_list)
    for cd in cores:
        g = len(cd["grp_n"])
        cd["grp_a"] = np.concatenate([cd["grp_a"], np.zeros(G - g, np.int64)])
        cd["grp_b"] = np.concatenate([cd["grp_b"], np.ones(G - g, np.int64)])
        cd["grp_n"] = np.concatenate([cd["grp_n"], np.zeros(G - g, np.int64)])
        r = len(cd["rt_codes"])
        par = np.ones(RT, np.int64)
        bb = np.zeros(RT, np.int64)
        par[:r] = cd["rt_codes"] >> 1
        bb[:r] = cd["rt_codes"] & 1
        cd["rt_par"] = par
        cd["rt_b"] = bb
    return cores, G, RT, perm


def kernel(primitives, identity, unique):
    global LAST_RESULTS
    from concourse.bass_utils import run_bass_kernel_spmd

    prims = np.ascontiguousarray(np.asarray(primitives, dtype=np.float32))
    u = np.asarray(unique).astype(np.int64).ravel()
    n = u.shape[0]
    assert n % NCORES == 0
    npos = n // NCORES

    cores, G, RT, perm = _pack(u, npos)
    eye = np.eye(DIM, dtype=np.float32)

    ckey = (npos, u.tobytes())
    if ckey not in _NC_CACHE:
        nc = _build_nc(npos, cores, G, RT)
        nc.compile()
        _NC_CACHE.clear()
        _NC_CACHE[ckey] = nc
    nc = _NC_CACHE[ckey]

    in_maps = [{"prims": prims, "ident": eye} for _ in range(NCORES)]

    import os
    trace_dir = os.environ.get("KERNEL_TRACE_DIR")
    res = run_bass_kernel_spmd(nc, in_maps, core_ids=list(range(NCORES)),
                               tmpdir=trace_dir)
    LAST_RESULTS = res

    parts = []
    for c in range(NCORES):
        o = np.asarray(res.results[c]["out"])
        o = o.reshape(npos, P, 2, DIM).transpose(0, 2, 1, 3)
        parts.append(o.reshape(npos, DIM, DIM).astype(np.float32))
    out = np.empty((n, DIM, DIM), np.float32)
    out[perm] = np.concatenate(parts, axis=0)

    ident = np.asarray(identity, dtype=np.float32)[0]
    if not np.allclose(ident, np.eye(DIM, dtype=np.float32)):
        out = np.einsum("ij,njk->nik", ident, out).astype(np.float32)
    return out


# revision 61
# speedup vs baseline: 1.2051x; 1.0050x over previous
"""Trainium2 Bass kernel for nn_BinaryPathEncoder — v2.

Math: 6+5+5 bit-chunk table decomposition.  maps[u] = A(u&63) @ B @ C where
A is the low-6-bit chunk (transposed table rt), B bits 6-10, C bits 11-15
(shared table rn).  Per position: mm1 = (A@B)^T (deduped over distinct
(A,B) keys, arbitrary group sizes), mm2 = out = (A@B) @ C.

v2 over baseline:
 - mm1 dedup with arbitrary group sizes (G ~= 163/core vs 190)
 - rt table per-core dense slots: only ~10 of 64 entries built per core
   (positions are sorted by (A,B) key so each core sees a narrow A band)
 - PE warmup matmuls during input-DMA head (HAM warm before expm)
 - expm scaling-squaring s=2 (one fewer squaring round)
 - transposes wait on DMA sem directly (not the DVE ident chain)
 - output copies 4-positions wide on ACT (amortize fixed overhead),
   output DMA per quad; redundant semaphore waits elided
"""

import contextlib
import numpy as np

DIM = 256
NCORES = 8
P = 128

ENT = 512          # f16 elems per table entry per partition (2*DIM)
NAT_E = 63         # rn: codes 1..63 (5-bit variable chunk products)
NAT_STRIDE = NAT_E * ENT

NSTAGE = 4         # mm1 psum banks (psA)
NSX = 8            # stag slots
LAG = 4            # min groups of mm1 lead before a group's mm2s start
DEADLINE = 8       # group h's mm2s must be emitted before mm1(h+DEADLINE)
EXPM_S = 2
EXPM_N = 4
WARMUP_MM = 20

_NC_CACHE = {}
LAST_RESULTS = None


class _WaitTracker:
    """Elide semaphore waits whose threshold is already guaranteed."""

    def __init__(self):
        self.guar = {}

    def wait(self, eng, sem, thr):
        if thr <= 0:
            return
        if self.guar.get(id(sem), -1) >= thr:
            return
        eng.wait_ge(sem, thr)
        self.guar[id(sem)] = thr


def _build_nc(npos, cores, G, RT, debug=False):
    """cores: per-core dict with keys:
         rt_par: [RT] parent rn code for each dense rt slot (1-padded)
         rt_b:   [RT] primitive index for each rt build (0-padded)
         grp_a:  [G] dense rt slot (0 = identity) per group (pad: 0)
         grp_b:  [G] rn code per group (pad: 1)
         grp_n:  [G] #positions per group (pad: 0)
         qC:     [npos] rn code for each position (emission order)
       G, RT: uniform (padded) group/rt-build counts, G even."""
    from concourse import bass, bacc, mybir

    f32 = mybir.dt.float32
    f32r = mybir.dt.float32r
    f16 = mybir.dt.float16
    Sub = mybir.AluOpType.subtract
    Add = mybir.AluOpType.add

    RT_STRIDE = (1 + RT) * ENT

    nc = bacc.Bacc("TRN2", target_bir_lowering=False, debug=debug)

    prims_ext = nc.dram_tensor("prims", [2, DIM, DIM], f32, kind="ExternalInput")
    ident_ext = nc.dram_tensor("ident", [DIM, DIM], f32, kind="ExternalInput")
    assert npos % 4 == 0
    nquad = npos // 4
    out_ext = nc.dram_tensor("out", [npos, P, 2 * DIM], f16, kind="ExternalOutput")

    with contextlib.ExitStack() as ctx:
        sem = {}
        for name in ["id_sem", "pr0_sem", "pr1_sem",
                     "pe_sem", "dve_sem", "act_sem",
                     "mm1_sem", "mm2_sem", "dvex_sem",
                     "oqa_sem", "oqd_sem",
                     "dma_q0", "dma_q1"]:
            sem[name] = ctx.enter_context(nc.semaphore(name))

        rn = ctx.enter_context(nc.sbuf_tensor("rn", [P, NAT_STRIDE], f16))
        rt = ctx.enter_context(nc.sbuf_tensor("rt", [P, RT_STRIDE], f16))
        pbf = ctx.enter_context(nc.sbuf_tensor("pbf", [P, 2, 2, DIM], f16))
        identf = ctx.enter_context(nc.sbuf_tensor("identf", [P, 2, DIM], f32))
        identr = ctx.enter_context(nc.sbuf_tensor("identr", [P, 2, DIM], f32r))
        i6 = ctx.enter_context(nc.sbuf_tensor("i6", [P, 2, DIM], f32r))
        i2 = ctx.enter_context(nc.sbuf_tensor("i2", [P, 2, DIM], f32r))
        prim = ctx.enter_context(nc.sbuf_tensor("prim", [P, 2, 2, DIM], f32))
        an_ = [ctx.enter_context(nc.sbuf_tensor(f"an{b}", [P, 2, DIM], f32r))
               for b in range(2)]
        a24 = [ctx.enter_context(nc.sbuf_tensor(f"a24_{b}", [P, 2, DIM], f32r))
               for b in range(2)]
        ye = [ctx.enter_context(nc.sbuf_tensor(f"ye{b}", [P, 2, DIM], f32r))
              for b in range(2)]
        yt = [ctx.enter_context(nc.sbuf_tensor(f"yt{b}", [P, 2, DIM], f32r))
              for b in range(2)]
        stag_x = ctx.enter_context(nc.sbuf_tensor("stag_x", [P, NSX, 2, DIM], f16))
        outb = ctx.enter_context(nc.sbuf_tensor("outb", [P, 8, 2, DIM], f16))
        scr = ctx.enter_context(nc.sbuf_tensor("scr", [P, 2 * P], f16))
        psA = ctx.enter_context(nc.psum_tensor("psA", [P, NSTAGE, 2, DIM], f32))
        psB = ctx.enter_context(nc.psum_tensor("psB", [P, NSTAGE, 2, DIM], f32))

        def bank(k):
            # 8-bank rotation across psA (0-3) and psB (4-7)
            return (psA if k < 4 else psB)[:, k % 4, :, :]

        ident128 = identf[:, 0, 0:P]

        def ent3(q):
            return bass.AP(rn, (q - 1) * ENT, [[NAT_STRIDE, P], [DIM, 2], [1, DIM]])

        def ent3t(s):
            return bass.AP(rt, s * ENT, [[RT_STRIDE, P], [DIM, 2], [1, DIM]])

        cnt = {k: 0 for k in sem}
        entry_done = {}
        pe_prog, dve_prog, act_prog, sync_prog, gps_prog = [], [], [], [], []

        # ---------------- DMA in ----------------
        def s_in(s):
            s.dma_start(identf[:, 0, :], ident_ext[0:P, :]).then_inc(sem["id_sem"], 16)
            s.dma_start(identf[:, 1, :], ident_ext[P:2 * P, :]).then_inc(sem["id_sem"], 16)
            for b in range(2):
                s.dma_start(prim[:, b, :, :],
                            bass.AP(prims_ext, b * DIM * DIM,
                                    [[DIM, P], [P * DIM, 2], [1, DIM]]),
                            ).then_inc(sem[f"pr{b}_sem"], 16)
        sync_prog.append(s_in)

        # ---------------- PE warmup (HAM) ----------------
        # scr is uninitialized SBUF: the results are garbage written to a
        # psum bank that is fully overwritten (start=True) before first use.
        def p_warm(t):
            for _ in range(WARMUP_MM):
                t.matmul(psB[:, 3, 0, :], scr[:, 0:P], scr[:, 0:2 * P],
                         start=True, stop=True)
        pe_prog.append(p_warm)

        # ---------------- ACT ident prep ----------------
        def a_ident(a):
            a.wait_ge(sem["id_sem"], 32)
            a.mul(identr[:, :, :], identf[:, :, :], 1.0)
            a.mul(i6[:, :, :], identf[:, :, :], 1.0 / 6.0)
            a.mul(i2[:, :, :], identf[:, :, :],
                  0.5).then_inc(sem["act_sem"], 1)
            # table identity entries (needed much later, by builds/mm1)
            a.mul(ent3(1), identf[:, :, :], 1.0)
            a.mul(ent3t(0), identf[:, :, :], 1.0).then_inc(sem["act_sem"], 1)
        act_prog.append(a_ident)
        cnt["act_sem"] += 2
        iprep_act = 1           # i6/i2/identr ready (act_sem)
        ident_act = 2           # table ident entries ready (act_sem)

        # ---------------- expm ----------------
        inv2s = 1.0 / (2.0 ** EXPM_S)

        for b in range(2):
            def p_tr(t, b=b):
                t.wait_ge(sem[f"pr{b}_sem"], 16)
                if b == 0:
                    t.wait_ge(sem["id_sem"], 32)
                last = None
                for kc in range(2):
                    for mc in range(2):
                        last = t.transpose(
                            out=bank(b)[:, kc, mc * P:(mc + 1) * P],
                            in_=prim[:, b, mc, kc * P:(kc + 1) * P],
                            identity=ident128)
                last.then_inc(sem["pe_sem"], 1)
            pe_prog.append(p_tr)
            cnt["pe_sem"] += 1

        prep_done = {}
        prep_act = {}
        for b in range(2):
            def d_prep(d, b=b, w=b + 1):
                d.wait_ge(sem["pe_sem"], w)
                d.tensor_tensor(out=ye[b][:, :, :], in0=bank(b)[:, :, :],
                                in1=prim[:, b, :, :], op=Sub)
                d.drain()
                d.tensor_scalar_mul(an_[b][:, :, :], ye[b][:, :, :], inv2s)
                d.tensor_scalar_mul(a24[b][:, :, :], ye[b][:, :, :],
                                    -inv2s / 24.0).then_inc(sem["dve_sem"], 1)
            dve_prog.append(d_prep)
            cnt["dve_sem"] += 1
            prep_done[b] = cnt["dve_sem"]
            prep_act[b] = iprep_act

        chains = [(0, 0), (0, 1), (1, 0), (1, 1)]
        ybuf = {(b, s): (ye[b] if s == 0 else yt[b])
                for b in range(2) for s in (0, 1)}
        bank_of = {c: 2 + i for i, c in enumerate(chains)}

        def emit_mm_fused(t, bk, parts, inc=None):
            last = None
            for mc in range(2):
                ops = [(lh, rh, kc) for lh, rh in parts for kc in range(2)]
                for idx, (lh, rh, kc) in enumerate(ops):
                    last = t.matmul(bank(bk)[:, mc, :],
                                    lh[:, kc, mc * P:(mc + 1) * P],
                                    rh[:, kc, :],
                                    start=(idx == 0),
                                    stop=(idx == len(ops) - 1))
            if inc is not None:
                last.then_inc(sem[inc], 1)
            return last

        copy_done = {}
        mm_done = {}
        addend = [i6, i2, identr]

        def wait_cds(t, *cds):
            best = {}
            for sname, c in cds:
                best[sname] = max(best.get(sname, 0), c)
            for sname, c in best.items():
                t.wait_ge(sem[sname], c)

        def emit_psum_copy(bk, d0, d1, full, add_ident=False, b=0):
            """Copy psum bank bk -> full; b==1 plain copies go to ACT."""
            w = cnt["pe_sem"]
            if add_ident or b == 0:
                def d_c(d, bk=bk, full=full, w=w, add_ident=add_ident):
                    d.wait_ge(sem["pe_sem"], w)
                    if add_ident:
                        d.tensor_tensor(out=full, in0=bank(bk)[:, :, :],
                                        in1=identf[:, :, :],
                                        op=Add).then_inc(sem["dve_sem"], 1)
                    else:
                        d.tensor_copy(full, bank(bk)[:, :, :],
                                      ).then_inc(sem["dve_sem"], 1)
                dve_prog.append(d_c)
                cnt["dve_sem"] += 1
                return (("dve_sem", cnt["dve_sem"]),)

            def a_c(a, bk=bk, full=full, w=w):
                a.wait_ge(sem["pe_sem"], w)
                a.mul(full, bank(bk)[:, :, :],
                      1.0).then_inc(sem["act_sem"], 1)
            act_prog.append(a_c)
            cnt["act_sem"] += 1
            return (("act_sem", cnt["act_sem"]),)

        for step in range(3):
            for b in range(2):
                s = 0
                wd = ((("dve_sem", prep_done[b]),) if step == 0
                      else copy_done[(b, s)])

                def p_h(t, b=b, s=s, step=step, wd=wd):
                    wait_cds(t, *wd)
                    if step == 0:
                        t.wait_ge(sem["act_sem"], prep_act[b])
                    bk = bank_of[(b, s)]
                    main = ((an_[b], a24[b]) if step == 0
                            else (an_[b], ybuf[(b, s)]))
                    emit_mm_fused(t, bk,
                                  [main, (an_[b], addend[step])],
                                  inc="pe_sem")
                pe_prog.append(p_h)
                cnt["pe_sem"] += 1
                mm_done[(b, s)] = cnt["pe_sem"]

                copy_done[(b, s)] = emit_psum_copy(
                    bank_of[(b, s)], None, None,
                    full=ybuf[(b, s)][:, :, :],
                    add_ident=(step == 2), b=b)

        # yt[b] = ye[b]^T via matmul against identity: T4(-x) == T4(x)^T
        for b in range(2):
            def p_tt(t, b=b, w=copy_done[(b, 0)]):
                wait_cds(t, *w)
                emit_mm_fused(t, bank_of[(b, 1)],
                              [(ybuf[(b, 0)], identr)], inc="pe_sem")
            pe_prog.append(p_tt)
            cnt["pe_sem"] += 1
            mm_done[(b, 1)] = cnt["pe_sem"]

            copy_done[(b, 1)] = emit_psum_copy(
                bank_of[(b, 1)], None, None,
                full=ybuf[(b, 1)][:, :, :], b=b)

        for sq in range(EXPM_S):
            last_sq = (sq == EXPM_S - 1)
            # last round keeps the s=1 chains too: their products are
            # P_b^T = mats[b], copied straight into table entries rn[2+b]
            active = list(chains)
            for (b, s) in active:
                def p_sq(t, b=b, s=s,
                         w0=copy_done[(b, 0)], w1=copy_done[(b, 1)]):
                    wait_cds(t, *w0, *w1)
                    emit_mm_fused(t, bank_of[(b, s)],
                                  [(ybuf[(b, 1 - s)], ybuf[(b, s)])],
                                  inc="pe_sem")
                pe_prog.append(p_sq)
                cnt["pe_sem"] += 1
                mm_done[(b, s)] = cnt["pe_sem"]

            for (b, s) in active:
                if last_sq:
                    full = pbf[:, b, :, :] if s == 0 else ent3(2 + b)
                else:
                    full = ybuf[(b, s)][:, :, :]
                # both of this b's products must be in psum before the
                # ybuf they read gets overwritten
                saved = cnt["pe_sem"]
                cnt["pe_sem"] = (mm_done[(b, s)] if last_sq
                                 else max(mm_done[(b, 0)], mm_done[(b, 1)]))
                copy_done[(b, s)] = emit_psum_copy(bank_of[(b, s)], None,
                                                   None, full=full, b=b)
                cnt["pe_sem"] = saved

        expm_cds = tuple(copy_done[(0, 0)]) + tuple(copy_done[(1, 0)])
        # P_b^T copies double as table entries rn[2], rn[3]
        ent_p = {2 + b: copy_done[(b, 1)][0] for b in range(2)}

        # ---------------- rn table build (uniform) ----------------
        # rn[q] for q=2..63; parent rn[q>>1], primitive pbf[q&1].
        pe_wt = _WaitTracker()
        dve_wt = _WaitTracker()
        act_wt = _WaitTracker()
        bank_owner = {}
        entry_done[1] = ("act_sem", ident_act)
        entry_done[2] = ent_p[2]
        entry_done[3] = ent_p[3]

        nb_total = 62 + RT
        for j in range(2, 62):
            q = j + 2
            b = q & 1
            par = q >> 1
            bk = j % 8

            waits = []
            if j < 8:
                waits.extend(expm_cds)
            waits.append(entry_done[par])
            if bk in bank_owner:
                waits.append(bank_owner[bk])

            def p_build(t, b=b, par=par, bk=bk, waits=tuple(waits)):
                for s_, c_ in waits:
                    pe_wt.wait(t, sem[s_], c_)
                last = None
                for mc in range(2):
                    for kc in range(2):
                        last = t.matmul(bank(bk)[:, mc, :],
                                        pbf[:, b, kc, mc * P:(mc + 1) * P],
                                        ent3(par)[:, kc, :],
                                        start=(kc == 0), stop=(kc == 1))
                last.then_inc(sem["pe_sem"], 1)
            pe_prog.append(p_build)
            cnt["pe_sem"] += 1

            ceng = "dve_sem" if j % 2 == 0 else "act_sem"
            prog = dve_prog if j % 2 == 0 else act_prog
            wtr = dve_wt if j % 2 == 0 else act_wt

            def x_copy(e, q=q, bk=bk, w=cnt["pe_sem"], ceng=ceng, wtr=wtr):
                wtr.wait(e, sem["pe_sem"], w)
                if ceng == "dve_sem":
                    e.tensor_copy(ent3(q), bank(bk)[:, :, :],
                                  ).then_inc(sem[ceng], 1)
                else:
                    e.mul(ent3(q), bank(bk)[:, :, :],
                          1.0).then_inc(sem[ceng], 1)
            prog.append(x_copy)
            cnt[ceng] += 1
            entry_done[q] = (ceng, cnt[ceng])
            bank_owner[bk] = (ceng, cnt[ceng])

        # ---------------- per-core section (PE only) ----------------
        # rt builds (copies are uniform, below) + position loop.
        pe_base = cnt["pe_sem"]
        rt_bank_owner = dict(bank_owner)

        def p_core(t):
            pid = t.partition_id()
            for c in t.Switch(pid, NCORES):
                cd = cores[c]
                wt = _WaitTracker()
                # inherit uniform guarantees
                wt.guar = dict(pe_wt.guar)

                # --- rt builds: rt slot 1+j = rn[par]^T-form product ---
                for j in range(RT):
                    par = int(cd["rt_par"][j])
                    b = int(cd["rt_b"][j])
                    bk = (62 + j) % 8
                    s_, c_ = entry_done[par]
                    wt.wait(t, sem[s_], c_)
                    s_, c_ = rt_bank_owner[bk]
                    wt.wait(t, sem[s_], c_)
                    last = None
                    for mc in range(2):
                        for kc in range(2):
                            last = t.matmul(bank(bk)[:, mc, :],
                                            ent3(par)[:, kc, mc * P:(mc + 1) * P],
                                            pbf[:, b, kc, :],
                                            start=(kc == 0), stop=(kc == 1))
                    last.then_inc(sem["pe_sem"], 1)

                # --- position loop (paced mm1/mm2 interleave) ---
                wt.wait(t, sem["dve_sem"], bd_total)
                wt.wait(t, sem["act_sem"], ba_total)
                g_of = []
                for g in range(G):
                    g_of += [g] * int(cd["grp_n"][g])
                assert len(g_of) == npos

                def emit_mm1(g, cd=cd, t=t, wt=wt):
                    wt.wait(t, sem["dvex_sem"], (g - 4) // 2 + 1)
                    slotA = int(cd["grp_a"][g])
                    qB = int(cd["grp_b"][g])
                    last = None
                    for mc in range(2):
                        for kc in range(2):
                            last = t.matmul(
                                psA[:, g % 4, mc, :],
                                ent3(qB)[:, kc, mc * P:(mc + 1) * P],
                                ent3t(slotA)[:, kc, :],
                                start=(kc == 0), stop=(kc == 1))
                    last.then_inc(sem["mm1_sem"], 1)

                def emit_mm2(p, cd=cd, t=t, wt=wt, g_of=g_of):
                    h = g_of[p]
                    wt.wait(t, sem["dvex_sem"], h // 2 + 1)
                    if p >= 4:
                        wt.wait(t, sem["oqa_sem"], (p - 4) // 2 + 1)
                    qC = int(cd["qC"][p])
                    last = None
                    for mc in range(2):
                        for kc in range(2):
                            last = t.matmul(
                                psB[:, p % 4, mc, :],
                                stag_x[:, h % NSX, kc, mc * P:(mc + 1) * P],
                                ent3(qC)[:, kc, :],
                                start=(kc == 0), stop=(kc == 1))
                    last.then_inc(sem["mm2_sem"], 1)

                p = 0
                for g in range(G):
                    # deadline: stag slot of group g-DEADLINE is recycled
                    # by the copy following mm1(g) -- drain its mm2s first
                    while p < npos and g_of[p] <= g - DEADLINE:
                        emit_mm2(p)
                        p += 1
                    emit_mm1(g)
                    target = ((g + 1) * npos) // G
                    burst = 0
                    while (p < npos and p < target and g_of[p] <= g - LAG
                           and burst < 2):
                        emit_mm2(p)
                        p += 1
                        burst += 1
                while p < npos:
                    emit_mm2(p)
                    p += 1
        pe_prog.append(p_core)

        # uniform rt copies (dense slots; psum banks rotate uniformly)
        for j in range(RT):
            bk = (62 + j) % 8
            ceng = "dve_sem" if (62 + j) % 2 == 0 else "act_sem"
            prog = dve_prog if (62 + j) % 2 == 0 else act_prog
            wtr = dve_wt if (62 + j) % 2 == 0 else act_wt

            def rt_copy(e, j=j, bk=bk, w=pe_base + j + 1, ceng=ceng, wtr=wtr):
                wtr.wait(e, sem["pe_sem"], w)
                if ceng == "dve_sem":
                    e.tensor_copy(ent3t(1 + j), bank(bk)[:, :, :],
                                  ).then_inc(sem[ceng], 1)
                else:
                    e.mul(ent3t(1 + j), bank(bk)[:, :, :],
                          1.0).then_inc(sem[ceng], 1)
            prog.append(rt_copy)
            cnt[ceng] += 1
            bank_owner[bk] = (ceng, cnt[ceng])

        bd_total = cnt["dve_sem"]
        ba_total = cnt["act_sem"]

        # ---- ACT: stag pair copies (uniform) ----
        def a_pos(a):
            for j in range(G // 2):
                act_wt.wait(a, sem["mm1_sem"], 2 * j + 2)
                sl = (2 * j) % NSX
                bk = (2 * j) % 4
                a.mul(stag_x[:, sl:sl + 2, :, :],
                      psA[:, bk:bk + 2, :, :],
                      1.0).then_inc(sem["dvex_sem"], 1)
        act_prog.append(a_pos)

        # ---- DVE: output pair copies (uniform) ----
        # pair k = positions (2k, 2k+1): psB banks (2k%4, +1) -> outb
        # slots ((k%4)*2, +1); pair-granular DMA frees outb slots.
        def d_pos(d):
            for k in range(npos // 2):
                dve_wt.wait(d, sem["mm2_sem"], 2 * k + 2)
                if k >= 4:
                    dve_wt.wait(d, sem[f"dma_q{k % 2}"],
                                16 * ((k - 4) // 2 + 1))
                sl = (k % 4) * 2
                bk = (2 * k) % 4
                d.tensor_copy(outb[:, sl:sl + 2, :, :],
                              psB[:, bk:bk + 2, :, :],
                              ).then_inc(sem["oqa_sem"], 1)
        dve_prog.append(d_pos)

        # ---------------- Sync: output pair DMAs ----------------
        def s_pos(s):
            for k in range(npos // 2):
                s.wait_ge(sem["oqa_sem"], k + 1)
                sl = (k % 4) * 2
                dst = bass.AP(out_ext, 2 * k * P * 2 * DIM,
                              [[2 * DIM, P], [P * 2 * DIM, 2], [1, 2 * DIM]])
                s.dma_start(dst, outb[:, sl:sl + 2, :, :],
                            ).then_inc(sem[f"dma_q{k % 2}"], 16)
            s.wait_ge(sem["dma_q0"], 16 * (npos // 4))
            s.wait_ge(sem["dma_q1"], 16 * (npos // 4))
        sync_prog.append(s_pos)

        # ---------------- emit ----------------
        with nc.Block(no_gpsimd_drain=True) as block:
            @block.tensor
            def _(tensor):
                for fn in pe_prog:
                    fn(tensor)

            @block.vector
            def _(vector):
                for fn in dve_prog:
                    fn(vector)

            @block.scalar
            def _(scalar):
                for fn in act_prog:
                    fn(scalar)

            @block.sync
            def _(sync):
                for fn in sync_prog:
                    fn(sync)

            if gps_prog:
                @block.gpsimd
                def _(gpsimd):
                    for fn in gps_prog:
                        fn(gpsimd)

    return nc


def _host_indices(u):
    """u: (n,) int64 positions -> (idxA, idxB, idxC) int arrays."""
    u = u.astype(np.int64)
    blen = np.zeros_like(u)
    t = u.copy()
    while np.any(t > 0):
        blen = np.where(t > 0, blen + 1, blen)
        t >>= 1
    k = blen - 1
    tA = np.minimum(k, 6)
    idxA = (1 << tA) + (u & ((1 << tA) - 1))
    tB = np.clip(k - 6, 0, 5)
    idxB = (1 << tB) + ((u >> 6) & ((1 << tB) - 1))
    tC = np.clip(k - 11, 0, 5)
    idxC = (1 << tC) + ((u >> 11) & ((1 << tC) - 1))
    short = u < 64
    idxA = np.where(short, 1, idxA)
    idxB = np.where(short, u, idxB)
    assert idxA.max() < 128 and idxB.max() < 64 and idxC.max() < 64
    assert np.all((idxA == 1) | (idxA >= 64))
    return idxA, idxB, idxC


def _pack(u, npos):
    """Sort by (A,B) key, shard contiguously, per-core group structure."""
    n = len(u)
    idxA, idxB, idxC = _host_indices(u)
    key = idxA.astype(np.int64) * 64 + idxB
    order = np.argsort(key, kind="stable")

    cores = []
    G_list, RT_list = [], []
    perm = np.empty(n, np.int64)
    for c in range(NCORES):
        sl = slice(c * npos, (c + 1) * npos)
        o = order[sl]
        kk = key[o]
        qA, qB, qC = idxA[o], idxB[o], idxC[o]
        newg = np.ones(npos, bool)
        newg[1:] = kk[1:] != kk[:-1]
        starts = np.flatnonzero(newg)
        sizes = np.diff(np.append(starts, npos))
        # dense rt slots for this core's A codes (code 1 -> slot 0)
        acodes = np.unique(qA[starts])
        acodes = acodes[acodes > 1]
        slot_of = {1: 0}
        for j, q in enumerate(acodes):
            slot_of[int(q)] = 1 + j
        cores.append({
            "rt_codes": acodes,
            "grp_a": np.array([slot_of[int(q)] for q in qA[starts]], np.int64),
            "grp_b": qB[starts].astype(np.int64),
            "grp_n": sizes.astype(np.int64),
            "qC": qC.astype(np.int64),
        })
        G_list.append(len(starts))
        RT_list.append(len(acodes))
        perm[c * npos:(c + 1) * npos] = o

    G = max(G_list)
    G += G % 2  # even
    RT = max(RT_list)
    for cd in cores:
        g = len(cd["grp_n"])
        cd["grp_a"] = np.concatenate([cd["grp_a"], np.zeros(G - g, np.int64)])
        cd["grp_b"] = np.concatenate([cd["grp_b"], np.ones(G - g, np.int64)])
        cd["grp_n"] = np.concatenate([cd["grp_n"], np.zeros(G - g, np.int64)])
        r = len(cd["rt_codes"])
        par = np.ones(RT, np.int64)
        bb = np.zeros(RT, np.int64)
        par[:r] = cd["rt_codes"] >> 1
        bb[:r] = cd["rt_codes"] & 1
        cd["rt_par"] = par
        cd["rt_b"] = bb
    return cores, G, RT, perm


def kernel(primitives, identity, unique):
    global LAST_RESULTS
    from concourse.bass_utils import run_bass_kernel_spmd

    prims = np.ascontiguousarray(np.asarray(primitives, dtype=np.float32))
    u = np.asarray(unique).astype(np.int64).ravel()
    n = u.shape[0]
    assert n % NCORES == 0
    npos = n // NCORES

    cores, G, RT, perm = _pack(u, npos)
    eye = np.eye(DIM, dtype=np.float32)

    ckey = (npos, u.tobytes())
    if ckey not in _NC_CACHE:
        nc = _build_nc(npos, cores, G, RT)
        nc.compile()
        _NC_CACHE.clear()
        _NC_CACHE[ckey] = nc
    nc = _NC_CACHE[ckey]

    in_maps = [{"prims": prims, "ident": eye} for _ in range(NCORES)]

    import os
    trace_dir = os.environ.get("KERNEL_TRACE_DIR")
    res = run_bass_kernel_spmd(nc, in_maps, core_ids=list(range(NCORES)),
                               tmpdir=trace_dir)
    LAST_RESULTS = res

    parts = []
    for c in range(NCORES):
        o = np.asarray(res.results[c]["out"])
        o = o.reshape(npos, P, 2, DIM).transpose(0, 2, 1, 3)
        parts.append(o.reshape(npos, DIM, DIM).astype(np.float32))
    out = np.empty((n, DIM, DIM), np.float32)
    out[perm] = np.concatenate(parts, axis=0)

    ident = np.asarray(identity, dtype=np.float32)[0]
    if not np.allclose(ident, np.eye(DIM, dtype=np.float32)):
        out = np.einsum("ij,njk->nik", ident, out).astype(np.float32)
    return out
